# revision 8
# baseline (speedup 1.0000x reference)
"""DisplaceChannel kernel for Trainium2 (8 NeuronCores, Bass/Tile).

out = depthwise3x3(displace(inp, round(offset)), gaussian(offset - round(offset)))

Strategy (v6):
- Data-parallel over batch: 32 batches -> 4 per core.
- Positions packed 4 per tile (4 pos x 4 batch x 8 chan = 128 partitions),
  grouped by EQUAL integer x-offset (sorted by y-offset inside a group so
  the row-band union stays tight).
- y-displacement folded into the input DMA row placement; rows copied
  FULL-WIDTH so each channel transfer is one contiguous Hv*256B chunk.
- x-displacement folded into the x-conv access-pattern offsets (uniform
  within a group).
- Separable conv, engine split:
  * y-stage on the TensorEngine: 3 accumulating diagonal-stationary
    matmuls in float32r (1 cycle/row when chunk >= 256 cols) into PSUM,
    with the per-position scale wy_k * wx_mid folded into the diagonals;
    ScalarE copies PSUM -> SBUF T tile.
  * x-stage on ScalarE+VectorE as 2 scalar_tensor_tensor MACs with taps
    normalized by the middle tap (wx0/wx1, wx2/wx1); the middle-tap
    scale already lives in the y-stage diagonals, so no ACTIVATE muls
    are needed anywhere.
- Output HBM is pre-zeroed by the runtime; each position's nonzero row
  band is written full-width (contiguous), zeros in off-band columns.
"""
import os
import sys

import numpy as np

for _p in ("/opt/trn_rl_repo", "/root/.axon_site/_ro/trn_rl_repo"):
    if os.path.isdir(_p) and _p not in sys.path:
        sys.path.insert(0, _p)
        break

from contextlib import ExitStack

import concourse.bass as bass  # noqa: F401
import concourse.tile as tile
from concourse import bacc, mybir
from concourse.bass_utils import run_bass_kernel_spmd

H = 64
W = 64
B = 32
CHAN_PER_POS = 8
NUM_POS = 48
C = NUM_POS * CHAN_PER_POS
SIGMA = 0.5
NCORES = 8
BL = B // NCORES
POS_PER_GROUP = 4
F32 = mybir.dt.float32
F32R = mybir.dt.float32r

_cache = {}


def _geometry(offset):
    off_round = np.round(offset)  # round-half-even, matches jnp.round
    oxy = off_round.astype(np.int64)
    frac = (offset - off_round).astype(np.float32)

    coords = (np.arange(3, dtype=np.float32) - np.float32(1.0))
    dx = coords[None, :] + frac[:, 0:1]
    dy = coords[None, :] + frac[:, 1:2]
    inv = np.float32(1.0 / (2.0 * SIGMA * SIGMA))
    gx = np.exp(-(dx * dx) * inv).astype(np.float32)
    gy = np.exp(-(dy * dy) * inv).astype(np.float32)
    wx = gx / gx.sum(axis=1, keepdims=True)
    wy = gy / gy.sum(axis=1, keepdims=True)

    pos = {}
    for p in range(NUM_POS):
        ox, oy = int(oxy[p, 0]), int(oxy[p, 1])
        vy0, vy1 = max(0, oy), min(H, H + oy)
        vx0, vx1 = max(0, ox), min(W, W + ox)
        if vy1 <= vy0 or vx1 <= vx0:
            continue
        pos[p] = dict(
            p=p, ox=ox, oy=oy, vy0=vy0, vy1=vy1,
            sy0=vy0 - oy, sx0=vx0 - ox, wv=vx1 - vx0,
            by0=max(0, vy0 - 1), by1=min(H, vy1 + 1),
            bx0=max(0, vx0 - 1), bx1=min(W, vx1 + 1),
        )

    by_ox = {}
    for p, m in sorted(pos.items(), key=lambda kv: (kv[1]["ox"], kv[1]["oy"])):
        by_ox.setdefault(m["ox"], []).append(m)

    groups = []
    for ox in sorted(by_ox):
        # members are row-band-ALIGNED inside the tile (each band placed at
        # local row 0), so a group costs max(band), not the absolute union.
        # Group the biggest bands together to minimize sum-of-max.
        mem = sorted(by_ox[ox], key=lambda m: m["by0"] - m["by1"])
        for i in range(0, len(mem), POS_PER_GROUP):
            members = mem[i:i + POS_PER_GROUP]
            bg = max(m["by1"] - m["by0"] for m in members)
            sx0 = members[0]["sx0"]
            wv = members[0]["wv"]
            ud0 = max(0, sx0 - 2)
            ud1 = min(W, sx0 + wv + 2)
            groups.append(dict(
                members=members, ox=ox, bg=bg,
                sx0=sx0, wv=wv, ud0=ud0, ud1=ud1,
                bx0=members[0]["bx0"], bx1=members[0]["bx1"],
            ))

    ng = len(groups)
    # x-stage taps, normalized by the middle tap (its scale is folded into
    # the y-stage diagonals): per group 2 per-partition scalars.
    taps = np.zeros((128, max(ng, 1) * 2), dtype=np.float32)
    # y-stage diagonal stationaries (3 per group), scaled by wx middle tap.
    diags = np.zeros((128, ng * 3 * 128), dtype=np.float32)
    for g, grp in enumerate(groups):
        for i, m in enumerate(grp["members"]):
            p = m["p"]
            rows = slice(i * 32, (i + 1) * 32)
            taps[rows, g * 2 + 0] = wx[p, 0] / wx[p, 1]
            taps[rows, g * 2 + 1] = wx[p, 2] / wx[p, 1]
            for k in range(3):
                base = (g * 3 + k) * 128
                for q in range(i * 32, (i + 1) * 32):
                    diags[q, base + q] = wy[p, k] * wx[p, 1]
    return groups, taps, diags


def _build(groups, n_tap_cols, n_diag_cols):
    nc = bacc.Bacc("TRN2", target_bir_lowering=False, debug=False,
                   num_devices=NCORES)
    inp_d = nc.dram_tensor("inp", [BL, C, H, W], F32, kind="ExternalInput")
    taps_d = nc.dram_tensor("taps", [128, n_tap_cols], F32, kind="ExternalInput")
    diags_d = nc.dram_tensor("diags", [128, n_diag_cols], F32,
                             kind="ExternalInput")
    out_d = nc.dram_tensor("out", [BL, C, H, W], F32, kind="ExternalOutput")

    mult = mybir.AluOpType.mult
    add = mybir.AluOpType.add
    dma_ctr = [0]
    gpsimd_out = os.environ.get("KERNEL_GPSIMD_OUT", "") == "1"

    with tile.TileContext(nc) as tc:
        with ExitStack() as ctx:
            dpool = ctx.enter_context(tc.tile_pool(name="dpool", bufs=4))
            tpool = ctx.enter_context(tc.tile_pool(name="tpool", bufs=3))
            opool = ctx.enter_context(tc.tile_pool(name="opool", bufs=3))
            cpool = ctx.enter_context(tc.tile_pool(name="cpool", bufs=1))
            pspool = ctx.enter_context(
                tc.tile_pool(name="pspool", bufs=8, space="PSUM"))

            taps_t = cpool.tile([128, n_tap_cols], F32, tag="taps")
            nc.sync.dma_start(taps_t[:], taps_d.ap()[:, :])
            diags_t = cpool.tile([128, n_diag_cols], F32R, tag="diags")
            # split the (large) stationary load so group 0's diags arrive
            # quickly and the rest streams in behind the first data tiles
            ndc = n_diag_cols // 4
            for ci in range(4):
                c0, c1 = ci * ndc, (ci + 1) * ndc if ci < 3 else n_diag_cols
                nc.sync.dma_start(diags_t[:, c0:c1],
                                  diags_d.ap()[:, c0:c1].bitcast(F32R))

            def tap(g, k):
                return taps_t[:, g * 2 + k:g * 2 + k + 1]

            def dma(dst, src):
                eng = (nc.sync, nc.scalar)[dma_ctr[0] % 2]
                dma_ctr[0] += 1
                eng.dma_start(dst, src)

            def stage_y_pe(t3, d3, ud0, wd, bg, g):
                # T[:, r, 2+j] = sum_k diag(wy_k*wxm) @ D[:, r+k, ud0+j]
                rpc = max(1, 512 // wd)
                nchunks = -(-bg // rpc)
                rpc = -(-bg // nchunks)
                r = 0
                while r < bg:
                    nr = min(rpc, bg - r)
                    acc = pspool.tile([128, nr * wd], F32, tag="ps")
                    accv = acc[:].rearrange("q (a b) -> q a b", b=wd)
                    for k in range(3):
                        lhsT = diags_t[:, (g * 3 + k) * 128:
                                       (g * 3 + k + 1) * 128]
                        nc.tensor.matmul(
                            acc[:, 0:nr * wd],
                            lhsT,
                            d3[:, r + k:r + k + nr, ud0:ud0 + wd],
                            start=(k == 0), stop=(k == 2))
                    nc.scalar.copy(t3[:, r:r + nr, 2:2 + wd], accv[:, :, :])
                    r += nr

            def stage_x_dve(out_ap, in_aps, g):
                # out = (T0 * s0) + T1 ; out += T2 * s2
                nc.vector.scalar_tensor_tensor(
                    out_ap, in_aps[0], tap(g, 0), in_aps[1], mult, add)
                nc.vector.scalar_tensor_tensor(
                    out_ap, in_aps[2], tap(g, 1), out_ap, mult, add)

            for g, grp in enumerate(groups):
                bg = grp["bg"]
                drows = bg + 2
                ox = grp["ox"]
                sx0, wv, ud0, ud1 = grp["sx0"], grp["wv"], grp["ud0"], grp["ud1"]
                wd = ud1 - ud0
                wt = wd + 4
                bx0, bx1 = grp["bx0"], grp["bx1"]
                wb = bx1 - bx0

                d_t = dpool.tile([128, drows * W], F32R, tag="D")
                d3 = d_t[:].rearrange("q (r c) -> q r c", c=W)
                # memset can't target f32r; zero through a uint32 view
                d3u = d_t[:].bitcast(mybir.dt.uint32).rearrange(
                    "q (r c) -> q r c", c=W)
                # first slots: VectorE is idle at kernel start and its memset
                # is faster; gpsimd's serial memset chain was the ramp bubble
                mseng = nc.vector if g < 3 else nc.gpsimd
                mseng.memset(d3u[:, :, ud0:ud1], 0)

                # partition layout within a member: q = i*32 + ch*4 + b, so
                # the DMA can put the 8-value channel dim outermost (the SDMA
                # engine index follows the outermost AP dim -> 8 engines)
                for i, m in enumerate(grp["members"]):
                    hv = m["vy1"] - m["vy0"]
                    r0 = 1 + m["vy0"] - m["by0"]
                    q0 = i * 32
                    dst = d_t[q0:q0 + 32, r0 * W:(r0 + hv) * W]
                    src = inp_d.ap()[:, 8 * m["p"]:8 * m["p"] + 8,
                                     m["sy0"]:m["sy0"] + hv, :]
                    dma(dst,
                        src.rearrange("b ch r c -> ch b (r c)").bitcast(F32R))

                if sx0 > ud0:
                    nc.gpsimd.memset(d3u[:, :, ud0:sx0], 0)
                if ud1 > sx0 + wv:
                    nc.gpsimd.memset(d3u[:, :, sx0 + wv:ud1], 0)

                # y-conv on PE: T[tr, 2+j] = sum_ky wy[ky]*wxm * D[tr+ky, ud0+j]
                t_t = tpool.tile([128, bg * wt], F32, tag="T")
                t3 = t_t[:].rearrange("q (r c) -> q r c", c=wt)
                nc.gpsimd.memset(t3[:, :, 0:2], 0.0)
                nc.gpsimd.memset(t3[:, :, wt - 2:wt], 0.0)
                stage_y_pe(t3, d3, ud0, wd, bg, g)

                # x-conv on DVE: O[tr, x] = sum_kx wx[kx] * T[tr, x-ox+kx-1-ud0+2]
                o_t = opool.tile([128, bg * W], F32, tag="O")
                o3 = o_t[:].rearrange("q (r c) -> q r c", c=W)
                if bx0 > 0:
                    nc.gpsimd.memset(o3[:, :, 0:bx0], 0.0)
                if bx1 < W:
                    nc.gpsimd.memset(o3[:, :, bx1:W], 0.0)
                c0 = bx0 - ox - 1 - ud0 + 2
                odat = o3[:, :, bx0:bx1]
                stage_x_dve(odat,
                            [t3[:, :, c0 + k:c0 + k + wb] for k in range(3)],
                            g)

                for i, m in enumerate(grp["members"]):
                    r0, r1 = 0, m["by1"] - m["by0"]
                    q0 = i * 32
                    src = o_t[q0:q0 + 32, r0 * W:r1 * W]
                    dst = out_d.ap()[:, 8 * m["p"]:8 * m["p"] + 8,
                                     m["by0"]:m["by1"], :]
                    if gpsimd_out:
                        nc.gpsimd.dma_start(
                            dst.rearrange("b ch r c -> ch b (r c)"), src)
                    else:
                        dma(dst.rearrange("b ch r c -> ch b (r c)"), src)

    nc.compile()
    return nc


def kernel(inp, offset):
    inp = np.ascontiguousarray(inp, dtype=np.float32)
    offset = np.ascontiguousarray(offset, dtype=np.float32)
    assert inp.shape == (B, C, H, W), inp.shape

    key = offset.tobytes()
    if key not in _cache:
        groups, taps, diags = _geometry(offset)
        nc = _build(groups, taps.shape[1], diags.shape[1])
        _cache[key] = (nc, taps, diags)
    nc, taps, diags = _cache[key]

    in_maps = [{"inp": inp[c * BL:(c + 1) * BL], "taps": taps, "diags": diags}
               for c in range(NCORES)]
    trace = os.environ.get("KERNEL_TRACE", "") == "1"
    try:
        res = run_bass_kernel_spmd(nc, in_maps, core_ids=list(range(NCORES)),
                                   trace=trace)
    except ModuleNotFoundError:
        # NTFF profile hook unavailable; run untraced
        trace = False
        res = run_bass_kernel_spmd(nc, in_maps, core_ids=list(range(NCORES)),
                                   trace=False)
    if trace:
        print(f"HW exec time: {res.exec_time_ns} ns "
              f"(mean {res.mean_exec_time_ns})")
        kernel.last_exec_time_ns = res.exec_time_ns
    out = np.concatenate([res.results[c]["out"] for c in range(NCORES)],
                         axis=0)
    return out


# revision 11
# speedup vs baseline: 1.2335x; 1.2335x over previous
"""DisplaceChannel kernel for Trainium2 (8 NeuronCores, Bass/Tile).

out = depthwise3x3(displace(inp, round(offset)), gaussian(offset - round(offset)))

Strategy (v6):
- Data-parallel over batch: 32 batches -> 4 per core.
- Positions packed 4 per tile (4 pos x 4 batch x 8 chan = 128 partitions),
  grouped by EQUAL integer x-offset (sorted by y-offset inside a group so
  the row-band union stays tight).
- y-displacement folded into the input DMA row placement; rows copied
  FULL-WIDTH so each channel transfer is one contiguous Hv*256B chunk.
- x-displacement folded into the x-conv access-pattern offsets (uniform
  within a group).
- Separable conv, engine split:
  * y-stage on the TensorEngine: 3 accumulating diagonal-stationary
    matmuls in float32r (1 cycle/row when chunk >= 256 cols) into PSUM,
    with the per-position scale wy_k * wx_mid folded into the diagonals;
    ScalarE copies PSUM -> SBUF T tile.
  * x-stage on ScalarE+VectorE as 2 scalar_tensor_tensor MACs with taps
    normalized by the middle tap (wx0/wx1, wx2/wx1); the middle-tap
    scale already lives in the y-stage diagonals, so no ACTIVATE muls
    are needed anywhere.
- Output HBM is pre-zeroed by the runtime; each position's nonzero row
  band is written full-width (contiguous), zeros in off-band columns.
"""
import os
import sys

import numpy as np

for _p in ("/opt/trn_rl_repo", "/root/.axon_site/_ro/trn_rl_repo"):
    if os.path.isdir(_p) and _p not in sys.path:
        sys.path.insert(0, _p)
        break

from contextlib import ExitStack

import concourse.bass as bass  # noqa: F401
import concourse.tile as tile
from concourse import bacc, mybir
from concourse.bass_utils import run_bass_kernel_spmd

H = 64
W = 64
B = 32
CHAN_PER_POS = 8
NUM_POS = 48
C = NUM_POS * CHAN_PER_POS
SIGMA = 0.5
NCORES = 8
BL = B // NCORES
POS_PER_GROUP = 4
F32 = mybir.dt.float32
F32R = mybir.dt.float32r
BF16 = mybir.dt.bfloat16

_cache = {}


def _geometry(offset):
    off_round = np.round(offset)  # round-half-even, matches jnp.round
    oxy = off_round.astype(np.int64)
    frac = (offset - off_round).astype(np.float32)

    coords = (np.arange(3, dtype=np.float32) - np.float32(1.0))
    dx = coords[None, :] + frac[:, 0:1]
    dy = coords[None, :] + frac[:, 1:2]
    inv = np.float32(1.0 / (2.0 * SIGMA * SIGMA))
    gx = np.exp(-(dx * dx) * inv).astype(np.float32)
    gy = np.exp(-(dy * dy) * inv).astype(np.float32)
    wx = gx / gx.sum(axis=1, keepdims=True)
    wy = gy / gy.sum(axis=1, keepdims=True)

    pos = {}
    for p in range(NUM_POS):
        ox, oy = int(oxy[p, 0]), int(oxy[p, 1])
        vy0, vy1 = max(0, oy), min(H, H + oy)
        vx0, vx1 = max(0, ox), min(W, W + ox)
        if vy1 <= vy0 or vx1 <= vx0:
            continue
        pos[p] = dict(
            p=p, ox=ox, oy=oy, vy0=vy0, vy1=vy1,
            sy0=vy0 - oy, sx0=vx0 - ox, wv=vx1 - vx0,
            by0=max(0, vy0 - 1), by1=min(H, vy1 + 1),
            bx0=max(0, vx0 - 1), bx1=min(W, vx1 + 1),
        )

    by_ox = {}
    for p, m in sorted(pos.items(), key=lambda kv: (kv[1]["ox"], kv[1]["oy"])):
        by_ox.setdefault(m["ox"], []).append(m)

    groups = []
    for ox in sorted(by_ox):
        # members are row-band-ALIGNED inside the tile (each band placed at
        # local row 0), so a group costs max(band), not the absolute union.
        # Group the biggest bands together to minimize sum-of-max.
        mem = sorted(by_ox[ox], key=lambda m: m["by0"] - m["by1"])
        for i in range(0, len(mem), POS_PER_GROUP):
            members = mem[i:i + POS_PER_GROUP]
            bg = max(m["by1"] - m["by0"] for m in members)
            sx0 = members[0]["sx0"]
            wv = members[0]["wv"]
            ud0 = max(0, sx0 - 2)
            ud1 = min(W, sx0 + wv + 2)
            groups.append(dict(
                members=members, ox=ox, bg=bg,
                sx0=sx0, wv=wv, ud0=ud0, ud1=ud1,
                bx0=members[0]["bx0"], bx1=members[0]["bx1"],
            ))

    ng = len(groups)
    # x-stage taps, normalized by the middle tap (its scale is folded into
    # the y-stage diagonals): per group 2 per-partition scalars.
    taps = np.zeros((128, max(ng, 1) * 2), dtype=np.float32)
    # y-stage diagonal stationaries (3 per group), scaled by wx middle tap.
    diags = np.zeros((128, ng * 3 * 128), dtype=np.float32)
    for g, grp in enumerate(groups):
        for i, m in enumerate(grp["members"]):
            p = m["p"]
            rows = slice(i * 32, (i + 1) * 32)
            taps[rows, g * 2 + 0] = wx[p, 0] / wx[p, 1]
            taps[rows, g * 2 + 1] = wx[p, 2] / wx[p, 1]
            for k in range(3):
                base = (g * 3 + k) * 128
                for q in range(i * 32, (i + 1) * 32):
                    diags[q, base + q] = wy[p, k] * wx[p, 1]
    return groups, taps, diags


def _build(groups, n_tap_cols, n_diag_cols):
    nc = bacc.Bacc("TRN2", target_bir_lowering=False, debug=False,
                   num_devices=NCORES)
    inp_d = nc.dram_tensor("inp", [BL, C, H, W], F32, kind="ExternalInput")
    taps_d = nc.dram_tensor("taps", [128, n_tap_cols], F32, kind="ExternalInput")
    diags_d = nc.dram_tensor("diags", [128, n_diag_cols], F32,
                             kind="ExternalInput")
    out_d = nc.dram_tensor("out", [BL, C, H, W], F32, kind="ExternalOutput")

    mult = mybir.AluOpType.mult
    add = mybir.AluOpType.add
    dma_ctr = [0]
    gpsimd_out = os.environ.get("KERNEL_GPSIMD_OUT", "") == "1"

    with tile.TileContext(nc) as tc:
        with ExitStack() as ctx:
            dpool = ctx.enter_context(tc.tile_pool(name="dpool", bufs=4))
            tpool = ctx.enter_context(tc.tile_pool(name="tpool", bufs=3))
            opool = ctx.enter_context(tc.tile_pool(name="opool", bufs=3))
            cpool = ctx.enter_context(tc.tile_pool(name="cpool", bufs=1))
            pspool = ctx.enter_context(
                tc.tile_pool(name="pspool", bufs=8, space="PSUM"))

            taps_t = cpool.tile([128, n_tap_cols], F32, tag="taps")
            nc.sync.dma_start(taps_t[:], taps_d.ap()[:, :])
            if os.environ.get("KERNEL_CAST_PROBE", "") == "1":
                cast_t = cpool.tile([128, n_tap_cols], BF16, tag="castprobe")
                nc.gpsimd.dma_start(cast_t[:], taps_d.ap()[:, :])
            diags_t = cpool.tile([128, n_diag_cols], F32R, tag="diags")
            # split the (large) stationary load so group 0's diags arrive
            # quickly and the rest streams in behind the first data tiles
            ndc = n_diag_cols // 4
            for ci in range(4):
                c0, c1 = ci * ndc, (ci + 1) * ndc if ci < 3 else n_diag_cols
                nc.sync.dma_start(diags_t[:, c0:c1],
                                  diags_d.ap()[:, c0:c1].bitcast(F32R))

            def tap(g, k):
                return taps_t[:, g * 2 + k:g * 2 + k + 1]

            def dma(dst, src):
                eng = (nc.sync, nc.scalar)[dma_ctr[0] % 2]
                dma_ctr[0] += 1
                eng.dma_start(dst, src)

            def stage_y_pe(t3, d3, ud0, wd, bg, g):
                # T[:, r, 2+j] = sum_k diag(wy_k*wxm) @ D[:, r+k, ud0+j]
                rpc = max(1, 512 // wd)
                nchunks = -(-bg // rpc)
                rpc = -(-bg // nchunks)
                r = 0
                while r < bg:
                    nr = min(rpc, bg - r)
                    acc = pspool.tile([128, nr * wd], F32, tag="ps")
                    accv = acc[:].rearrange("q (a b) -> q a b", b=wd)
                    for k in range(3):
                        lhsT = diags_t[:, (g * 3 + k) * 128:
                                       (g * 3 + k + 1) * 128]
                        nc.tensor.matmul(
                            acc[:, 0:nr * wd],
                            lhsT,
                            d3[:, r + k:r + k + nr, ud0:ud0 + wd],
                            start=(k == 0), stop=(k == 2))
                    nc.scalar.copy(t3[:, r:r + nr, 2:2 + wd], accv[:, :, :])
                    r += nr

            def stage_x_dve(out_ap, in_aps, g):
                # out = (T0 * s0) + T1 ; out += T2 * s2
                nc.vector.scalar_tensor_tensor(
                    out_ap, in_aps[0], tap(g, 0), in_aps[1], mult, add)
                nc.vector.scalar_tensor_tensor(
                    out_ap, in_aps[2], tap(g, 1), out_ap, mult, add)

            for g, grp in enumerate(groups):
                bg = grp["bg"]
                drows = bg + 2
                ox = grp["ox"]
                sx0, wv, ud0, ud1 = grp["sx0"], grp["wv"], grp["ud0"], grp["ud1"]
                wd = ud1 - ud0
                wt = wd + 4
                bx0, bx1 = grp["bx0"], grp["bx1"]
                wb = bx1 - bx0

                d_t = dpool.tile([128, drows * W], F32R, tag="D")
                d3 = d_t[:].rearrange("q (r c) -> q r c", c=W)
                # memset can't target f32r; zero through a uint32 view
                d3u = d_t[:].bitcast(mybir.dt.uint32).rearrange(
                    "q (r c) -> q r c", c=W)
                # first slots: VectorE is idle at kernel start and its memset
                # is faster; gpsimd's serial memset chain was the ramp bubble
                mseng = nc.vector if g < 3 else nc.gpsimd
                mseng.memset(d3u[:, :, ud0:ud1], 0)

                # partition layout within a member: q = i*32 + ch*4 + b, so
                # the DMA can put the 8-value channel dim outermost (the SDMA
                # engine index follows the outermost AP dim -> 8 engines)
                for i, m in enumerate(grp["members"]):
                    hv = m["vy1"] - m["vy0"]
                    r0 = 1 + m["vy0"] - m["by0"]
                    q0 = i * 32
                    dst = d_t[q0:q0 + 32, r0 * W:(r0 + hv) * W]
                    src = inp_d.ap()[:, 8 * m["p"]:8 * m["p"] + 8,
                                     m["sy0"]:m["sy0"] + hv, :]
                    dma(dst,
                        src.rearrange("b ch r c -> ch b (r c)").bitcast(F32R))

                if sx0 > ud0:
                    nc.gpsimd.memset(d3u[:, :, ud0:sx0], 0)
                if ud1 > sx0 + wv:
                    nc.gpsimd.memset(d3u[:, :, sx0 + wv:ud1], 0)

                # y-conv on PE: T[tr, 2+j] = sum_ky wy[ky]*wxm * D[tr+ky, ud0+j]
                t_t = tpool.tile([128, bg * wt], F32, tag="T")
                t3 = t_t[:].rearrange("q (r c) -> q r c", c=wt)
                nc.gpsimd.memset(t3[:, :, 0:2], 0.0)
                nc.gpsimd.memset(t3[:, :, wt - 2:wt], 0.0)
                stage_y_pe(t3, d3, ud0, wd, bg, g)

                # x-conv on DVE: O[tr, x] = sum_kx wx[kx] * T[tr, x-ox+kx-1-ud0+2]
                o_t = opool.tile([128, bg * W], F32, tag="O")
                o3 = o_t[:].rearrange("q (r c) -> q r c", c=W)
                if bx0 > 0:
                    nc.gpsimd.memset(o3[:, :, 0:bx0], 0.0)
                if bx1 < W:
                    nc.gpsimd.memset(o3[:, :, bx1:W], 0.0)
                c0 = bx0 - ox - 1 - ud0 + 2
                odat = o3[:, :, bx0:bx1]
                stage_x_dve(odat,
                            [t3[:, :, c0 + k:c0 + k + wb] for k in range(3)],
                            g)

                for i, m in enumerate(grp["members"]):
                    r0, r1 = 0, m["by1"] - m["by0"]
                    q0 = i * 32
                    src = o_t[q0:q0 + 32, r0 * W:r1 * W]
                    dst = out_d.ap()[:, 8 * m["p"]:8 * m["p"] + 8,
                                     m["by0"]:m["by1"], :]
                    if gpsimd_out:
                        nc.gpsimd.dma_start(
                            dst.rearrange("b ch r c -> ch b (r c)"), src)
                    else:
                        dma(dst.rearrange("b ch r c -> ch b (r c)"), src)

    nc.compile()
    return nc


def kernel(inp, offset):
    inp = np.ascontiguousarray(inp, dtype=np.float32)
    offset = np.ascontiguousarray(offset, dtype=np.float32)
    assert inp.shape == (B, C, H, W), inp.shape

    key = offset.tobytes()
    if key not in _cache:
        groups, taps, diags = _geometry(offset)
        nc = _build(groups, taps.shape[1], diags.shape[1])
        _cache[key] = (nc, taps, diags)
    nc, taps, diags = _cache[key]

    in_maps = [{"inp": inp[c * BL:(c + 1) * BL], "taps": taps, "diags": diags}
               for c in range(NCORES)]
    trace = os.environ.get("KERNEL_TRACE", "") == "1"
    try:
        res = run_bass_kernel_spmd(nc, in_maps, core_ids=list(range(NCORES)),
                                   trace=trace)
    except ModuleNotFoundError:
        # NTFF profile hook unavailable; run untraced
        trace = False
        res = run_bass_kernel_spmd(nc, in_maps, core_ids=list(range(NCORES)),
                                   trace=False)
    if trace:
        print(f"HW exec time: {res.exec_time_ns} ns "
              f"(mean {res.mean_exec_time_ns})")
        kernel.last_exec_time_ns = res.exec_time_ns
    out = np.concatenate([res.results[c]["out"] for c in range(NCORES)],
                         axis=0)
    return out
